# revision 20
# baseline (speedup 1.0000x reference)
import sys

import numpy as np

sys.path.insert(0, "/opt/trn_rl_repo")

import concourse.bass as bass  # noqa: F401
import concourse.mybir as mybir
import concourse.tile as tile
from concourse import bacc
from concourse.bass_utils import run_bass_kernel_spmd

D = H = W = 128
SIGMA = 3
K = 7
N_CORES = 8

HC = 8   # h rows per inbound DMA chunk (0.5 MiB f32)
GC = 16  # d' columns per outbound group

_NC_CACHE = {}


def _blur_matrix(g: np.ndarray) -> np.ndarray:
    # Dense 128x128 operator for a clamped (edge-padded) 1D blur along a
    # length-128 axis: A[i, j] = sum of g[k] over taps where clamp(i+k-3)==j.
    A = np.zeros((D, D), dtype=np.float64)
    for i in range(D):
        for k in range(K):
            j = min(max(i + k - SIGMA, 0), D - 1)
            A[i, j] += float(g[k])
    return A


def _build():
    nc = bacc.Bacc("TRN2", target_bir_lowering=False, debug=True)
    x = nc.dram_tensor("x", [D, H, W], mybir.dt.float32, kind="ExternalInput")
    at = nc.dram_tensor("at", [D, D], mybir.dt.float16, kind="ExternalInput")
    out = nc.dram_tensor("out", [D, H, W], mybir.dt.float32, kind="ExternalOutput")

    f16 = mybir.dt.float16
    f32 = mybir.dt.float32

    with tile.TileContext(nc) as tc:
        with tc.tile_pool(name="big", bufs=1) as big, \
             tc.tile_pool(name="cst", bufs=1) as cst, \
             tc.tile_pool(name="sout", bufs=4) as sout, \
             tc.tile_pool(name="pst", bufs=5, space="PSUM") as pst, \
             tc.tile_pool(name="pss", bufs=3, space="PSUM") as pss:
            att = cst.tile([D, D], f16)
            nc.sync.dma_start(att[:], at[:])

            xh = big.tile([D, H * W], f16)   # (d, h*128 + w)
            yt = big.tile([D, H * W], f16)   # (w, d'*128 + h)
            zt = big.tile([D, H * W], f16)   # (h, d'*128 + w')
            # view of Y as (w, d', h) for the P1 evacuation dst
            y3 = yt[:].rearrange("w (d h) -> w d h", h=H)
            out_v = out[:].rearrange("d h w -> h d w")

            ci = 0

            def evac(dst, src):
                nonlocal ci
                if ci % 2 == 0:
                    nc.vector.tensor_copy(dst, src)
                else:
                    nc.scalar.copy(dst, src)
                ci += 1

            # ---- Phase A: DMA-in (cast f32->f16 in SWDGE) + P1 (blur D, transpose) ----
            for c in range(H // HC):
                nc.gpsimd.dma_start(
                    xh[:, c * HC * W:(c + 1) * HC * W],
                    x[:, c * HC:(c + 1) * HC, :].rearrange("d h w -> d (h w)"))
                for gi in range(HC // 4):
                    pt = pst.tile([D, 512], f32)
                    h0 = c * HC + gi * 4
                    for j in range(4):
                        h = h0 + j
                        # two 64-col halves: LDW of one col-group overlaps the
                        # other col-group's matmul (per-subarray concurrency)
                        for cg in range(2):
                            nc.tensor.matmul(
                                pt[cg * 64:(cg + 1) * 64, j * 128:(j + 1) * 128],
                                xh[:, h * 128 + cg * 64:h * 128 + (cg + 1) * 64],
                                att[:], start=True, stop=True,
                                tile_position=(0, cg * 64))
                    # enumerate (d' outer, h inner): strided f32 PSUM reads,
                    # 4x2B contiguous write runs into Y's (d'*128 + h) layout
                    evac(y3[:, :, h0:h0 + 4],
                         pt[:].rearrange("w (h d) -> w d h", h=4))

            # ---- Phase B/C: P2 (blur W, transpose) + P3 (blur H) + DMA-out ----
            for g in range(D // GC):
                for k in range(GC // 4):
                    pt = pst.tile([D, 512], f32)
                    d0 = g * GC + k * 4
                    for j in range(4):
                        dd = d0 + j
                        for cg in range(2):
                            nc.tensor.matmul(
                                pt[cg * 64:(cg + 1) * 64, j * 128:(j + 1) * 128],
                                yt[:, dd * 128 + cg * 64:dd * 128 + (cg + 1) * 64],
                                att[:], start=True, stop=True,
                                tile_position=(0, cg * 64))
                    evac(zt[:, d0 * 128:d0 * 128 + 512], pt[:])
                for half in range(2):
                    so = sout.tile([D, GC * 64], f32)
                    for k in range(GC * 128 // 1024):
                        ps = pss.tile([D, 512], f32)
                        n0 = g * GC * 128 + half * GC * 64 + k * 512
                        nc.tensor.matmul(ps[:], att[:], zt[:, n0:n0 + 512],
                                         start=True, stop=True)
                        evac(so[:, k * 512:(k + 1) * 512], ps[:])
                    d0 = g * GC + half * (GC // 2)
                    nc.sync.dma_start(out_v[:, d0:d0 + GC // 2, :], so[:])
    nc.finalize()
    return nc


def kernel(x, g, sigma):
    x = np.ascontiguousarray(np.asarray(x, dtype=np.float32))
    g = np.asarray(g, dtype=np.float64)
    key = tuple(float(v) for v in g)
    if key not in _NC_CACHE:
        _NC_CACHE[key] = _build()
    nc = _NC_CACHE[key]
    AT = np.ascontiguousarray(_blur_matrix(g).T.astype(np.float16))
    slabs = x.reshape(N_CORES, D, H, W)
    in_maps = [{"x": np.ascontiguousarray(slabs[i]), "at": AT} for i in range(N_CORES)]
    res = run_bass_kernel_spmd(nc, in_maps, core_ids=list(range(N_CORES)))
    global LAST_RESULT
    LAST_RESULT = res
    outs = np.stack([res.results[i]["out"] for i in range(N_CORES)])
    return outs.reshape(2, 4, D, H, W).astype(np.float32)


LAST_RESULT = None


# revision 22
# speedup vs baseline: 1.1065x; 1.1065x over previous
import sys

import numpy as np

sys.path.insert(0, "/opt/trn_rl_repo")

import concourse.bass as bass  # noqa: F401
import concourse.mybir as mybir
import concourse.tile as tile
from concourse import bacc
from concourse.bass_utils import run_bass_kernel_spmd

D = H = W = 128
SIGMA = 3
K = 7
N_CORES = 8

HC = 8   # h rows per inbound DMA chunk (0.5 MiB f32)
GC = 16  # d' columns per outbound group

_NC_CACHE = {}


def _blur_matrix(g: np.ndarray) -> np.ndarray:
    # Dense 128x128 operator for a clamped (edge-padded) 1D blur along a
    # length-128 axis: A[i, j] = sum of g[k] over taps where clamp(i+k-3)==j.
    A = np.zeros((D, D), dtype=np.float64)
    for i in range(D):
        for k in range(K):
            j = min(max(i + k - SIGMA, 0), D - 1)
            A[i, j] += float(g[k])
    return A


def _build():
    nc = bacc.Bacc("TRN2", target_bir_lowering=False, debug=True)
    x = nc.dram_tensor("x", [D, H, W], mybir.dt.float32, kind="ExternalInput")
    at = nc.dram_tensor("at", [D, D], mybir.dt.float16, kind="ExternalInput")
    out = nc.dram_tensor("out", [D, H, W], mybir.dt.float32, kind="ExternalOutput")

    f16 = mybir.dt.float16
    f32 = mybir.dt.float32

    with tile.TileContext(nc) as tc:
        with tc.tile_pool(name="big", bufs=1) as big, \
             tc.tile_pool(name="cst", bufs=1) as cst, \
             tc.tile_pool(name="sout", bufs=3) as sout, \
             tc.tile_pool(name="pst", bufs=3, space="PSUM") as pst, \
             tc.tile_pool(name="pss", bufs=2, space="PSUM") as pss:
            att = cst.tile([D, D], f16)
            nc.sync.dma_start(att[:], at[:])

            xh = big.tile([D, H * W], f16)   # (d, h*128 + w)
            yt = big.tile([D, H * W], f16)   # (w, d'*128 + h)
            zt = big.tile([D, H * W], f16)   # (h, d'*128 + w')
            # view of Y as (w, d', h) for the P1 evacuation dst
            y3 = yt[:].rearrange("w (d h) -> w d h", h=H)
            out_v = out[:].rearrange("d h w -> h d w")

            ci = 0

            def evac(dst, src):
                nonlocal ci
                if ci % 2 == 0:
                    nc.vector.tensor_copy(dst, src)
                else:
                    nc.scalar.copy(dst, src)
                ci += 1

            # ---- Phase A: DMA-in (cast f32->f16 in SWDGE) + P1 (blur D, transpose) ----
            for c in range(H // HC):
                nc.gpsimd.dma_start(
                    xh[:, c * HC * W:(c + 1) * HC * W],
                    x[:, c * HC:(c + 1) * HC, :].rearrange("d h w -> d (h w)"))
                for gi in range(HC // 8):
                    pt = pst.tile([D, 1024], f32)
                    h0 = c * HC + gi * 8
                    for j in range(8):
                        h = h0 + j
                        nc.tensor.matmul(pt[:, j * 128:(j + 1) * 128],
                                         xh[:, h * 128:(h + 1) * 128], att[:],
                                         start=True, stop=True)
                    # enumerate (d' outer, h inner): strided f32 PSUM reads,
                    # 8x2B contiguous write runs into Y's (d'*128 + h) layout
                    evac(y3[:, :, h0:h0 + 8],
                         pt[:].rearrange("w (h d) -> w d h", h=8))

            # ---- Phase B/C: P2 (blur W, transpose) + P3 (blur H) + DMA-out ----
            for g in range(D // GC):
                for k in range(GC // 8):
                    pt = pst.tile([D, 1024], f32)
                    d0 = g * GC + k * 8
                    for j in range(8):
                        dd = d0 + j
                        nc.tensor.matmul(pt[:, j * 128:(j + 1) * 128],
                                         yt[:, dd * 128:(dd + 1) * 128], att[:],
                                         start=True, stop=True)
                    evac(zt[:, d0 * 128:d0 * 128 + 1024], pt[:])
                for half in range(2):
                    so = sout.tile([D, GC * 64], f32)
                    for k in range(GC * 128 // 1024):
                        ps = pss.tile([D, 512], f32)
                        n0 = g * GC * 128 + half * GC * 64 + k * 512
                        nc.tensor.matmul(ps[:], att[:], zt[:, n0:n0 + 512],
                                         start=True, stop=True)
                        evac(so[:, k * 512:(k + 1) * 512], ps[:])
                    d0 = g * GC + half * (GC // 2)
                    if g == D // GC - 1 and half == 1:
                        # split the last store so the tail-exposed DMA is small
                        q = GC // 4
                        nc.sync.dma_start(out_v[:, d0:d0 + q, :], so[:, :q * 128])
                        nc.sync.dma_start(out_v[:, d0 + q:d0 + 2 * q, :],
                                          so[:, q * 128:])
                    else:
                        nc.sync.dma_start(out_v[:, d0:d0 + GC // 2, :], so[:])
    nc.finalize()
    return nc


def kernel(x, g, sigma):
    x = np.ascontiguousarray(np.asarray(x, dtype=np.float32))
    g = np.asarray(g, dtype=np.float64)
    key = tuple(float(v) for v in g)
    if key not in _NC_CACHE:
        _NC_CACHE[key] = _build()
    nc = _NC_CACHE[key]
    AT = np.ascontiguousarray(_blur_matrix(g).T.astype(np.float16))
    slabs = x.reshape(N_CORES, D, H, W)
    in_maps = [{"x": np.ascontiguousarray(slabs[i]), "at": AT} for i in range(N_CORES)]
    res = run_bass_kernel_spmd(nc, in_maps, core_ids=list(range(N_CORES)))
    global LAST_RESULT
    LAST_RESULT = res
    outs = np.stack([res.results[i]["out"] for i in range(N_CORES)])
    return outs.reshape(2, 4, D, H, W).astype(np.float32)


LAST_RESULT = None
